# revision 27
# baseline (speedup 1.0000x reference)
"""Trainium2 Bass kernel for nn_NearestMemoryManager.

Reference computation (B=8, n_pos=1024, n_neg=5, D=128, nLem=20000):
  similarity       = einsum('bpd,md->bpm', x_pos, memory)          [8,1024,20000]
  noise_similarity = einsum('bnd,md->bnm', x_neg, memory[:1024])   [8,5,1024]
  get              = segment_sum(x_pos * visible, y) / B           [1024,128]
  new_memory       = l2norm(concat(memory[:1024]*m + get*(1-m),
                                   memory[1024:] w/ x_neg inserted at lru slot))

Sharding (8 cores):
  - similarity: memory-bank columns (nLem) sharded, 2500 per core; every core
    computes all batches for its slab.
  - noise_similarity: first-1024-rows sharded, 128 columns per core.
  - get/EMA: segment-sharded, 128 segments per core, computed via one-hot
    matmul scatter over all B*n_pos positions (no collective needed).
  - tail rows (1024..20000 with x_neg insertion done host-side as pure data
    movement): row-slab sharded, 2372 rows per core, l2-normalized on HW.
"""

import numpy as np

import concourse.bass as bass
import concourse.tile as tile
from concourse import bacc, mybir
from concourse.bass_utils import run_bass_kernel_spmd

B = 8
N_POS = 1024
N_NEG = 5
D = 128
N_LEM = 20000
NCORES = 8
MOMENTUM = 0.5
EPS = 1e-12

SLAB = N_LEM // NCORES            # 2500 similarity columns per core
NSUB = 5                          # matmuls per stripe (N=500 each)
NSUB_W = SLAB // NSUB             # 500
NM = (B * N_POS) // 128           # 64 output stripes of 128 rows
NKCH = (B * N_POS) // 128         # 64 contraction chunks for get
N_TAIL = N_LEM - N_POS            # 18976
TAIL_PER = N_TAIL // NCORES       # 2372
TAIL_CH = 19                      # chunks of 128 rows (padded)
TAIL_PAD = TAIL_CH * 128          # 2432

_F32 = mybir.dt.float32
# float32r: same 4-byte layout as float32, but the PE streams it at 1
# cycle/row (vs 4 for float32) when the moving free dim is >= 256.
_SIM_MM_DT = mybir.dt.float32r

_prog_cache = {}


def _l2norm_rows(nc, src_ap, out_ap, sq_pool, small_pool):
    """out = src / max(sqrt(sum(src*src, axis=free)), EPS) per partition row."""
    p = src_ap.shape[0]
    sq = sq_pool.tile([128, 128], _F32, tag="sq")
    ss = small_pool.tile([128, 1], _F32, tag="ss")
    nc.scalar.activation(
        out=sq[:p],
        in_=src_ap,
        func=mybir.ActivationFunctionType.Square,
        accum_out=ss[:p],
    )
    nrm = small_pool.tile([128, 1], _F32, tag="nrm")
    nc.scalar.sqrt(nrm[:p], ss[:p])
    dn = small_pool.tile([128, 1], _F32, tag="dn")
    nc.vector.tensor_scalar_max(dn[:p], nrm[:p], EPS)
    rcp = small_pool.tile([128, 1], _F32, tag="rcp")
    nc.vector.reciprocal(rcp[:p], dn[:p])
    nc.vector.tensor_scalar_mul(out_ap, src_ap, rcp[:p])


def _build_program():
    nc = bacc.Bacc("TRN2", target_bir_lowering=False, debug=False)

    # -- inputs (per-core) --
    memT = nc.dram_tensor("memT", [128, SLAB], _F32, kind="ExternalInput").ap()
    xT = nc.dram_tensor("xT", [128, B * N_POS], _F32, kind="ExternalInput").ap()
    ycols = nc.dram_tensor("ycols", [128, NKCH], _F32, kind="ExternalInput").ap()
    vcols = nc.dram_tensor("vcols", [128, NKCH], _F32, kind="ExternalInput").ap()
    iota = nc.dram_tensor("iota", [128, 128], _F32, kind="ExternalInput").ap()
    ident8 = nc.dram_tensor("ident8", [128, 128], _F32, kind="ExternalInput").ap()
    ident1 = nc.dram_tensor("ident1", [128, 128], _F32, kind="ExternalInput").ap()
    mempos = nc.dram_tensor("mempos", [128, 128], _F32, kind="ExternalInput").ap()
    memTn = nc.dram_tensor("memTn", [128, 128], _F32, kind="ExternalInput").ap()
    xnegT = nc.dram_tensor("xnegT", [128, B * N_NEG], _F32, kind="ExternalInput").ap()
    tail = nc.dram_tensor("tail", [TAIL_PAD, 128], _F32, kind="ExternalInput").ap()

    # -- outputs (per-core) --
    sim = nc.dram_tensor("sim", [B * N_POS, SLAB], _F32, kind="ExternalOutput").ap()
    noise = nc.dram_tensor("noise", [B * N_NEG, 128], _F32, kind="ExternalOutput").ap()
    nmpos = nc.dram_tensor("nmpos", [128, 128], _F32, kind="ExternalOutput").ap()
    nmtail = nc.dram_tensor("nmtail", [TAIL_PAD, 128], _F32, kind="ExternalOutput").ap()

    with tile.TileContext(nc) as tc:
        with (
            tc.tile_pool(name="resident", bufs=1) as resident,
            tc.tile_pool(name="consts", bufs=1) as consts,
            tc.tile_pool(name="stage", bufs=6) as stage,
            tc.tile_pool(name="onehot", bufs=4) as onehot,
            tc.tile_pool(name="tailio", bufs=4) as tailio,
            tc.tile_pool(name="sq", bufs=2) as sq_pool,
            tc.tile_pool(name="small", bufs=6) as small_pool,
            tc.tile_pool(name="ps_sim", bufs=6, space="PSUM") as ps_sim,
            tc.tile_pool(name="ps_get", bufs=1, space="PSUM") as ps_get,
            tc.tile_pool(name="ps_misc", bufs=1, space="PSUM") as ps_misc,
        ):
            # resident loads
            memT_sb = resident.tile([128, SLAB], _F32, tag="memT")
            nc.sync.dma_start(out=memT_sb[:], in_=memT)
            xT_sb = resident.tile([128, B * N_POS], _F32, tag="xT")
            for q in range(4):
                s = bass.ts(q, (B * N_POS) // 4)
                nc.sync.dma_start(out=xT_sb[:, s], in_=xT[:, s])
            # fp32r-rounded copies for the PE (1 cycle/row vs 4 for fp32)
            memT_r = resident.tile([128, SLAB], _SIM_MM_DT, tag="memT_r")
            for q in range(2):
                s = bass.ts(q, SLAB // 2)
                nc.vector.tensor_copy(out=memT_r[:, s], in_=memT_sb[:, s])
            xT_r = resident.tile([128, B * N_POS], _SIM_MM_DT, tag="xT_r")
            for q in range(4):
                s = bass.ts(q, (B * N_POS) // 4)
                nc.vector.tensor_copy(out=xT_r[:, s], in_=xT_sb[:, s])
            iota_sb = consts.tile([128, 128], _F32, tag="iota")
            nc.sync.dma_start(out=iota_sb[:], in_=iota)
            ident8_sb = consts.tile([128, 128], _F32, tag="ident8")
            nc.sync.dma_start(out=ident8_sb[:], in_=ident8)
            ident1_sb = consts.tile([128, 128], _F32, tag="ident1")
            nc.sync.dma_start(out=ident1_sb[:], in_=ident1)
            ycols_sb = consts.tile([128, NKCH], _F32, tag="ycols")
            nc.sync.dma_start(out=ycols_sb[:], in_=ycols)
            vcols_sb = consts.tile([128, NKCH], _F32, tag="vcols")
            nc.sync.dma_start(out=vcols_sb[:], in_=vcols)
            mempos_sb = consts.tile([128, 128], _F32, tag="mempos")
            nc.sync.dma_start(out=mempos_sb[:], in_=mempos)
            memTn_sb = consts.tile([128, 128], _F32, tag="memTn")
            nc.sync.dma_start(out=memTn_sb[:], in_=memTn)
            xnegT_sb = consts.tile([128, B * N_NEG], _F32, tag="xnegT")
            nc.sync.dma_start(out=xnegT_sb[:], in_=xnegT)

            # noise similarity: [40, 128] = x_negT.T @ memTn
            psn = ps_misc.tile([128, 128], _F32, tag="psm")
            nc.tensor.matmul(
                out=psn[: B * N_NEG],
                lhsT=xnegT_sb[:],
                rhs=memTn_sb[:],
                start=True,
                stop=True,
            )
            noise_sb = consts.tile([B * N_NEG, 128], _F32, tag="noise_sb")
            nc.vector.tensor_copy(out=noise_sb[:], in_=psn[: B * N_NEG])
            nc.sync.dma_start(out=noise, in_=noise_sb[:])

            # get accumulator (segment-sharded one-hot scatter matmul)
            psg = ps_get.tile([128, 128], _F32, tag="psg")

            tail_re = tail.rearrange("(n p) d -> n p d", p=128)
            nmtail_re = nmtail.rearrange("(n p) d -> n p d", p=128)

            def emit_tail_chunk(t):
                t_in = tailio.tile([128, 128], _F32, tag="t_in")
                nc.sync.dma_start(out=t_in[:], in_=tail_re[t])
                t_out = tailio.tile([128, 128], _F32, tag="t_out")
                _l2norm_rows(nc, t_in[:], t_out[:], sq_pool, small_pool)
                nc.sync.dma_start(out=nmtail_re[t], in_=t_out[:])

            # main loop: 64 similarity stripes, one get-chunk interleaved each.
            # Each stripe is staged in two tiles so the first-half DMA can
            # launch while the second half is still being copied out of PSUM.
            for m in range(NM):
                st_a = stage.tile([128, 3 * NSUB_W], _F32, tag="stripeA")
                st_b = stage.tile([128, 2 * NSUB_W], _F32, tag="stripeB")
                for j in range(NSUB):
                    ps = ps_sim.tile([128, NSUB_W], _F32, tag="ps")
                    nc.tensor.matmul(
                        out=ps[:],
                        lhsT=xT_r[:, bass.ts(m, 128)],
                        rhs=memT_r[:, bass.ts(j, NSUB_W)],
                        start=True,
                        stop=True,
                    )
                    if j < 3:
                        dst = st_a[:, bass.ts(j, NSUB_W)]
                    else:
                        dst = st_b[:, bass.ts(j - 3, NSUB_W)]
                    if j in (0, 3):
                        nc.scalar.copy(dst, ps[:])
                    else:
                        nc.vector.tensor_copy(out=dst, in_=ps[:])
                    if j == 2:
                        nc.sync.dma_start(
                            out=sim[bass.ts(m, 128), 0 : 3 * NSUB_W], in_=st_a[:]
                        )
                nc.sync.dma_start(
                    out=sim[bass.ts(m, 128), 3 * NSUB_W : SLAB], in_=st_b[:]
                )

                # one get contraction chunk: O = (iota == y_k), Xv = x_k * vis_k
                # x_k natural layout comes from a PE transpose of the xT chunk.
                k = m
                o_t = onehot.tile([128, 128], _F32, tag="o")
                nc.vector.tensor_scalar(
                    out=o_t[:],
                    in0=iota_sb[:],
                    scalar1=ycols_sb[:, k : k + 1],
                    scalar2=None,
                    op0=mybir.AluOpType.is_equal,
                )
                pst = ps_misc.tile([128, 128], _F32, tag="psm")
                nc.tensor.transpose(
                    out=pst[:], in_=xT_sb[:, bass.ts(k, 128)], identity=ident1_sb[:]
                )
                xv_t = onehot.tile([128, 128], _F32, tag="xv")
                nc.vector.tensor_scalar(
                    out=xv_t[:],
                    in0=pst[:],
                    scalar1=vcols_sb[:, k : k + 1],
                    scalar2=None,
                    op0=mybir.AluOpType.mult,
                )
                nc.tensor.matmul(
                    out=psg[:], lhsT=o_t[:], rhs=xv_t[:], start=(k == 0), stop=False
                )

                # spread the 19 tail-row chunks across the main loop
                if m % 3 == 2 and m // 3 < TAIL_CH:
                    emit_tail_chunk(m // 3)

            # EMA: psg += 8 * mempos  (so 0.0625*psg = segsum/16 + mempos/2)
            nc.tensor.matmul(
                out=psg[:], lhsT=ident8_sb[:], rhs=mempos_sb[:], start=False, stop=True
            )
            nm_un = consts.tile([128, 128], _F32, tag="nm_un")
            nc.scalar.mul(nm_un[:], psg[:], (1.0 - MOMENTUM) / B)
            nm_out = consts.tile([128, 128], _F32, tag="nm_out")
            _l2norm_rows(nc, nm_un[:], nm_out[:], sq_pool, small_pool)
            nc.sync.dma_start(out=nmpos, in_=nm_out[:])

    nc.compile()
    return nc


def _get_program():
    if "nc" not in _prog_cache:
        _prog_cache["nc"] = _build_program()
    return _prog_cache["nc"]


def _make_in_maps(x, y, visible, memory, lru_i):
    x = np.asarray(x, dtype=np.float32)
    memory = np.asarray(memory, dtype=np.float32)
    x_pos = x[:, :N_POS, :]
    x_neg = x[:, N_POS:, :]

    xpos_flat = np.ascontiguousarray(x_pos.reshape(B * N_POS, D))
    xT = np.ascontiguousarray(xpos_flat.T)                      # [128, 8192]
    xnegT = np.ascontiguousarray(x_neg.reshape(B * N_NEG, D).T)  # [128, 40]
    memT = np.ascontiguousarray(memory.T)                        # [128, 20000]

    yf = np.asarray(y).astype(np.float32).reshape(-1)
    vf = np.asarray(visible).astype(np.float32).reshape(-1)
    ycols = np.ascontiguousarray(yf.reshape(NKCH, 128).T)        # [128, 64]
    vcols = np.ascontiguousarray(vf.reshape(NKCH, 128).T)
    ident8 = np.ascontiguousarray(np.eye(128, dtype=np.float32) * float(B))
    ident1 = np.ascontiguousarray(np.eye(128, dtype=np.float32))

    start = N_POS + lru_i * N_NEG * B
    tail_full = np.concatenate(
        [memory[N_POS:start], x_neg.reshape(-1, D), memory[start + N_NEG * B :]],
        axis=0,
    )                                                            # [18976, 128]
    pad = np.ones((TAIL_PAD - TAIL_PER, D), dtype=np.float32)

    in_maps = []
    for c in range(NCORES):
        iota_c = np.ascontiguousarray(
            np.tile(
                (np.arange(128, dtype=np.float32) + 128.0 * c)[None, :], (128, 1)
            )
        )
        tail_c = np.concatenate(
            [tail_full[c * TAIL_PER : (c + 1) * TAIL_PER], pad], axis=0
        )
        in_maps.append(
            {
                "memT": np.ascontiguousarray(memT[:, c * SLAB : (c + 1) * SLAB]),
                "xT": xT,
                "ycols": ycols,
                "vcols": vcols,
                "iota": iota_c,
                "ident8": ident8,
                "ident1": ident1,
                "mempos": np.ascontiguousarray(memory[c * 128 : (c + 1) * 128]),
                "memTn": np.ascontiguousarray(memT[:, c * 128 : (c + 1) * 128]),
                "xnegT": xnegT,
                "tail": np.ascontiguousarray(tail_c),
            }
        )
    return in_maps


def _assemble(results, y):
    similarity = np.concatenate(
        [r["sim"].reshape(B, N_POS, SLAB) for r in results], axis=2
    )
    noise = np.concatenate(
        [r["noise"].reshape(B, N_NEG, 128) for r in results], axis=2
    )
    new_memory = np.concatenate(
        [r["nmpos"] for r in results]
        + [r["nmtail"][:TAIL_PER] for r in results],
        axis=0,
    )
    y_idx = np.asarray(y).copy()
    return similarity, y_idx, noise, new_memory


def kernel(x, y, visible, memory, lru):
    lru_i = int(lru)
    nc = _get_program()
    in_maps = _make_in_maps(x, y, visible, memory, lru_i)
    res = run_bass_kernel_spmd(nc, in_maps, core_ids=list(range(NCORES)))
    return _assemble(res.results, y)


# revision 35
# speedup vs baseline: 1.0558x; 1.0558x over previous
"""Trainium2 Bass kernel for nn_NearestMemoryManager.

Reference computation (B=8, n_pos=1024, n_neg=5, D=128, nLem=20000):
  similarity       = einsum('bpd,md->bpm', x_pos, memory)          [8,1024,20000]
  noise_similarity = einsum('bnd,md->bnm', x_neg, memory[:1024])   [8,5,1024]
  get              = segment_sum(x_pos * visible, y) / B           [1024,128]
  new_memory       = l2norm(concat(memory[:1024]*m + get*(1-m),
                                   memory[1024:] w/ x_neg inserted at lru slot))

Sharding (8 cores):
  - similarity: memory-bank columns (nLem) sharded, 2500 per core; every core
    computes all batches for its slab.
  - noise_similarity: first-1024-rows sharded, 128 columns per core.
  - get/EMA: segment-sharded, 128 segments per core, computed via one-hot
    matmul scatter over all B*n_pos positions (no collective needed).
  - tail rows (1024..20000 with x_neg insertion done host-side as pure data
    movement): row-slab sharded, 2372 rows per core, l2-normalized on HW.
"""

import numpy as np

import concourse.bass as bass
import concourse.tile as tile
from concourse import bacc, mybir
from concourse.bass_utils import run_bass_kernel_spmd

B = 8
N_POS = 1024
N_NEG = 5
D = 128
N_LEM = 20000
NCORES = 8
MOMENTUM = 0.5
EPS = 1e-12

SLAB = N_LEM // NCORES            # 2500 similarity columns per core
NSUB = 5                          # matmuls per stripe (N=500 each)
NSUB_W = SLAB // NSUB             # 500
NM = (B * N_POS) // 128           # 64 output stripes of 128 rows
NKCH = (B * N_POS) // 128         # 64 contraction chunks for get
N_TAIL = N_LEM - N_POS            # 18976
TAIL_PER = N_TAIL // NCORES       # 2372
TAIL_CH = 19                      # chunks of 128 rows (padded)
TAIL_PAD = TAIL_CH * 128          # 2432

_F32 = mybir.dt.float32
# float32r: same 4-byte layout as float32, but the PE streams it at 1
# cycle/row (vs 4 for float32) when the moving free dim is >= 256.
_SIM_MM_DT = mybir.dt.float32r

_prog_cache = {}


def _l2norm_rows(nc, src_ap, out_ap, sq_pool, small_pool):
    """out = src / max(sqrt(sum(src*src, axis=free)), EPS) per partition row."""
    p = src_ap.shape[0]
    sq = sq_pool.tile([128, 128], _F32, tag="sq")
    ss = small_pool.tile([128, 1], _F32, tag="ss")
    nc.scalar.activation(
        out=sq[:p],
        in_=src_ap,
        func=mybir.ActivationFunctionType.Square,
        accum_out=ss[:p],
    )
    nrm = small_pool.tile([128, 1], _F32, tag="nrm")
    nc.scalar.sqrt(nrm[:p], ss[:p])
    dn = small_pool.tile([128, 1], _F32, tag="dn")
    nc.vector.tensor_scalar_max(dn[:p], nrm[:p], EPS)
    rcp = small_pool.tile([128, 1], _F32, tag="rcp")
    nc.vector.reciprocal(rcp[:p], dn[:p])
    nc.vector.tensor_scalar_mul(out_ap, src_ap, rcp[:p])


def _build_program():
    nc = bacc.Bacc("TRN2", target_bir_lowering=False, debug=False)

    # -- inputs (per-core) --
    memT = nc.dram_tensor("memT", [128, SLAB], _F32, kind="ExternalInput").ap()
    xT = nc.dram_tensor("xT", [128, B * N_POS], _F32, kind="ExternalInput").ap()
    ycols = nc.dram_tensor("ycols", [128, NKCH], _F32, kind="ExternalInput").ap()
    vcols = nc.dram_tensor("vcols", [128, NKCH], _F32, kind="ExternalInput").ap()
    iota = nc.dram_tensor("iota", [128, 128], _F32, kind="ExternalInput").ap()
    ident8 = nc.dram_tensor("ident8", [128, 128], _F32, kind="ExternalInput").ap()
    ident1 = nc.dram_tensor("ident1", [128, 128], _F32, kind="ExternalInput").ap()
    mempos = nc.dram_tensor("mempos", [128, 128], _F32, kind="ExternalInput").ap()
    memTn = nc.dram_tensor("memTn", [128, 128], _F32, kind="ExternalInput").ap()
    xnegT = nc.dram_tensor("xnegT", [128, B * N_NEG], _F32, kind="ExternalInput").ap()
    # tail rows packed host-side as [p, t*128+d] so the load is one
    # fully-contiguous DMA (9.5KB/partition lines) instead of 19 512B-line DMAs
    tailp = nc.dram_tensor("tailp", [128, TAIL_CH * 128], _F32, kind="ExternalInput").ap()

    # -- outputs (per-core) --
    sim = nc.dram_tensor("sim", [B * N_POS, SLAB], _F32, kind="ExternalOutput").ap()
    noise = nc.dram_tensor("noise", [B * N_NEG, 128], _F32, kind="ExternalOutput").ap()
    nmpos = nc.dram_tensor("nmpos", [128, 128], _F32, kind="ExternalOutput").ap()
    nmtailp = nc.dram_tensor(
        "nmtailp", [128, TAIL_CH * 128], _F32, kind="ExternalOutput"
    ).ap()

    with tile.TileContext(nc) as tc:
        with (
            tc.tile_pool(name="resident", bufs=1) as resident,
            tc.tile_pool(name="consts", bufs=1) as consts,
            tc.tile_pool(name="stage", bufs=6) as stage,
            tc.tile_pool(name="onehot", bufs=4) as onehot,
            tc.tile_pool(name="tailio", bufs=4) as tailio,
            tc.tile_pool(name="sq", bufs=2) as sq_pool,
            tc.tile_pool(name="small", bufs=6) as small_pool,
            tc.tile_pool(name="ps_sim", bufs=6, space="PSUM") as ps_sim,
            tc.tile_pool(name="ps_get", bufs=1, space="PSUM") as ps_get,
            tc.tile_pool(name="ps_misc", bufs=1, space="PSUM") as ps_misc,
        ):
            # resident loads
            memT_sb = resident.tile([128, SLAB], _F32, tag="memT")
            nc.sync.dma_start(out=memT_sb[:], in_=memT)
            xT_sb = resident.tile([128, B * N_POS], _F32, tag="xT")
            for q in range(4):
                s = bass.ts(q, (B * N_POS) // 4)
                nc.sync.dma_start(out=xT_sb[:, s], in_=xT[:, s])
            # fp32r-rounded copies for the PE (1 cycle/row vs 4 for fp32)
            memT_r = resident.tile([128, SLAB], _SIM_MM_DT, tag="memT_r")
            for q in range(2):
                s = bass.ts(q, SLAB // 2)
                nc.vector.tensor_copy(out=memT_r[:, s], in_=memT_sb[:, s])
            xT_r = resident.tile([128, B * N_POS], _SIM_MM_DT, tag="xT_r")
            for q in range(4):
                s = bass.ts(q, (B * N_POS) // 4)
                nc.vector.tensor_copy(out=xT_r[:, s], in_=xT_sb[:, s])
            iota_sb = consts.tile([128, 128], _F32, tag="iota")
            nc.sync.dma_start(out=iota_sb[:], in_=iota)
            ident8_sb = consts.tile([128, 128], _F32, tag="ident8")
            nc.sync.dma_start(out=ident8_sb[:], in_=ident8)
            ident1_sb = consts.tile([128, 128], _F32, tag="ident1")
            nc.sync.dma_start(out=ident1_sb[:], in_=ident1)
            ycols_sb = consts.tile([128, NKCH], _F32, tag="ycols")
            nc.sync.dma_start(out=ycols_sb[:], in_=ycols)
            vcols_sb = consts.tile([128, NKCH], _F32, tag="vcols")
            nc.sync.dma_start(out=vcols_sb[:], in_=vcols)
            mempos_sb = consts.tile([128, 128], _F32, tag="mempos")
            nc.sync.dma_start(out=mempos_sb[:], in_=mempos)
            memTn_sb = consts.tile([128, 128], _F32, tag="memTn")
            nc.sync.dma_start(out=memTn_sb[:], in_=memTn)
            xnegT_sb = consts.tile([128, B * N_NEG], _F32, tag="xnegT")
            nc.sync.dma_start(out=xnegT_sb[:], in_=xnegT)

            # noise similarity: [40, 128] = x_negT.T @ memTn
            psn = ps_misc.tile([128, 128], _F32, tag="psm")
            nc.tensor.matmul(
                out=psn[: B * N_NEG],
                lhsT=xnegT_sb[:],
                rhs=memTn_sb[:],
                start=True,
                stop=True,
            )
            noise_sb = consts.tile([B * N_NEG, 128], _F32, tag="noise_sb")
            nc.vector.tensor_copy(out=noise_sb[:], in_=psn[: B * N_NEG])
            nc.sync.dma_start(out=noise, in_=noise_sb[:])

            # get accumulator (segment-sharded one-hot scatter matmul)
            psg = ps_get.tile([128, 128], _F32, tag="psg")

            # tail rows: one big load, per-chunk l2norm, one big store
            tail_sb = resident.tile([128, TAIL_CH * 128], _F32, tag="tail_sb")
            for q in range(2):
                s = bass.ds(q * 10 * 128, (10 - q) * 128)
                nc.sync.dma_start(out=tail_sb[:, s], in_=tailp[:, s])
            ntail_sb = resident.tile([128, TAIL_CH * 128], _F32, tag="ntail_sb")

            def emit_tail_chunk(t):
                s = bass.ts(t, 128)
                _l2norm_rows(nc, tail_sb[:, s], ntail_sb[:, s], sq_pool, small_pool)
                if t == TAIL_CH - 1:
                    for q in range(2):
                        sq_ = bass.ds(q * 10 * 128, (10 - q) * 128)
                        nc.sync.dma_start(out=nmtailp[:, sq_], in_=ntail_sb[:, sq_])

            # main loop: 64 similarity stripes, one get-chunk interleaved each.
            # Each stripe is staged in two tiles so the first-half DMA can
            # launch while the second half is still being copied out of PSUM.
            for m in range(NM):
                st_a = stage.tile([128, 3 * NSUB_W], _F32, tag="stripeA")
                st_b = stage.tile([128, 2 * NSUB_W], _F32, tag="stripeB")
                for j in range(NSUB):
                    ps = ps_sim.tile([128, NSUB_W], _F32, tag="ps")
                    nc.tensor.matmul(
                        out=ps[:],
                        lhsT=xT_r[:, bass.ts(m, 128)],
                        rhs=memT_r[:, bass.ts(j, NSUB_W)],
                        start=True,
                        stop=True,
                    )
                    if j < 3:
                        dst = st_a[:, bass.ts(j, NSUB_W)]
                    else:
                        dst = st_b[:, bass.ts(j - 3, NSUB_W)]
                    if j in (2, 4):
                        nc.scalar.copy(dst, ps[:])
                    else:
                        nc.vector.tensor_copy(out=dst, in_=ps[:])
                    if j == 2:
                        nc.sync.dma_start(
                            out=sim[bass.ts(m, 128), 0 : 3 * NSUB_W], in_=st_a[:]
                        )
                nc.sync.dma_start(
                    out=sim[bass.ts(m, 128), 3 * NSUB_W : SLAB], in_=st_b[:]
                )

                # one get contraction chunk: O = (iota == y_k), Xv = x_k * vis_k
                # x_k natural layout comes from a PE transpose of the xT chunk.
                k = m
                o_t = onehot.tile([128, 128], _F32, tag="o")
                nc.vector.tensor_scalar(
                    out=o_t[:],
                    in0=iota_sb[:],
                    scalar1=ycols_sb[:, k : k + 1],
                    scalar2=None,
                    op0=mybir.AluOpType.is_equal,
                )
                pst = ps_misc.tile([128, 128], _F32, tag="psm")
                nc.tensor.transpose(
                    out=pst[:], in_=xT_sb[:, bass.ts(k, 128)], identity=ident1_sb[:]
                )
                xv_t = onehot.tile([128, 128], _F32, tag="xv")
                nc.vector.tensor_scalar(
                    out=xv_t[:],
                    in0=pst[:],
                    scalar1=vcols_sb[:, k : k + 1],
                    scalar2=None,
                    op0=mybir.AluOpType.mult,
                )
                nc.tensor.matmul(
                    out=psg[:], lhsT=o_t[:], rhs=xv_t[:], start=(k == 0), stop=False
                )

                # spread the 19 tail-row chunks across the main loop
                if m % 3 == 2 and m // 3 < TAIL_CH:
                    emit_tail_chunk(m // 3)

            # EMA: psg += 8 * mempos  (so 0.0625*psg = segsum/16 + mempos/2)
            nc.tensor.matmul(
                out=psg[:], lhsT=ident8_sb[:], rhs=mempos_sb[:], start=False, stop=True
            )
            nm_un = consts.tile([128, 128], _F32, tag="nm_un")
            nc.scalar.mul(nm_un[:], psg[:], (1.0 - MOMENTUM) / B)
            nm_out = consts.tile([128, 128], _F32, tag="nm_out")
            _l2norm_rows(nc, nm_un[:], nm_out[:], sq_pool, small_pool)
            nc.sync.dma_start(out=nmpos, in_=nm_out[:])

    nc.compile()
    return nc


def _get_program():
    if "nc" not in _prog_cache:
        _prog_cache["nc"] = _build_program()
    return _prog_cache["nc"]


def _make_in_maps(x, y, visible, memory, lru_i):
    x = np.asarray(x, dtype=np.float32)
    memory = np.asarray(memory, dtype=np.float32)
    x_pos = x[:, :N_POS, :]
    x_neg = x[:, N_POS:, :]

    xpos_flat = np.ascontiguousarray(x_pos.reshape(B * N_POS, D))
    xT = np.ascontiguousarray(xpos_flat.T)                      # [128, 8192]
    xnegT = np.ascontiguousarray(x_neg.reshape(B * N_NEG, D).T)  # [128, 40]
    memT = np.ascontiguousarray(memory.T)                        # [128, 20000]

    yf = np.asarray(y).astype(np.float32).reshape(-1)
    vf = np.asarray(visible).astype(np.float32).reshape(-1)
    ycols = np.ascontiguousarray(yf.reshape(NKCH, 128).T)        # [128, 64]
    vcols = np.ascontiguousarray(vf.reshape(NKCH, 128).T)
    ident8 = np.ascontiguousarray(np.eye(128, dtype=np.float32) * float(B))
    ident1 = np.ascontiguousarray(np.eye(128, dtype=np.float32))

    start = N_POS + lru_i * N_NEG * B
    tail_full = np.concatenate(
        [memory[N_POS:start], x_neg.reshape(-1, D), memory[start + N_NEG * B :]],
        axis=0,
    )                                                            # [18976, 128]
    pad = np.ones((TAIL_PAD - TAIL_PER, D), dtype=np.float32)

    in_maps = []
    for c in range(NCORES):
        iota_c = np.ascontiguousarray(
            np.tile(
                (np.arange(128, dtype=np.float32) + 128.0 * c)[None, :], (128, 1)
            )
        )
        tail_c = np.concatenate(
            [tail_full[c * TAIL_PER : (c + 1) * TAIL_PER], pad], axis=0
        )
        tailp_c = np.ascontiguousarray(
            tail_c.reshape(TAIL_CH, 128, D).transpose(1, 0, 2).reshape(128, TAIL_CH * D)
        )
        in_maps.append(
            {
                "memT": np.ascontiguousarray(memT[:, c * SLAB : (c + 1) * SLAB]),
                "xT": xT,
                "ycols": ycols,
                "vcols": vcols,
                "iota": iota_c,
                "ident8": ident8,
                "ident1": ident1,
                "mempos": np.ascontiguousarray(memory[c * 128 : (c + 1) * 128]),
                "memTn": np.ascontiguousarray(memT[:, c * 128 : (c + 1) * 128]),
                "xnegT": xnegT,
                "tailp": tailp_c,
            }
        )
    return in_maps


def _assemble(results, y):
    similarity = np.concatenate(
        [r["sim"].reshape(B, N_POS, SLAB) for r in results], axis=2
    )
    noise = np.concatenate(
        [r["noise"].reshape(B, N_NEG, 128) for r in results], axis=2
    )
    new_memory = np.concatenate(
        [r["nmpos"] for r in results]
        + [
            r["nmtailp"]
            .reshape(128, TAIL_CH, D)
            .transpose(1, 0, 2)
            .reshape(TAIL_PAD, D)[:TAIL_PER]
            for r in results
        ],
        axis=0,
    )
    y_idx = np.asarray(y).copy()
    return similarity, y_idx, noise, new_memory


def kernel(x, y, visible, memory, lru):
    lru_i = int(lru)
    nc = _get_program()
    in_maps = _make_in_maps(x, y, visible, memory, lru_i)
    res = run_bass_kernel_spmd(nc, in_maps, core_ids=list(range(NCORES)))
    return _assemble(res.results, y)
